# revision 30
# baseline (speedup 1.0000x reference)
"""Trainium2 Bass kernel for nn_MeanAggregator (GNN mean aggregation).

out[b] = relu(concat(features[node[b]], mean_k features[neighbours[b,k]]) @ W)

8 NeuronCores, data-parallel over the batch (4096 items/core).  Tolerance is
2e-2, so features/W are cast to bf16 on the host (measured end-to-end error
~2.5e-3).

Gather strategy: the only SWDGE path whose Q7 descriptor generation is
vectorized is `dma_gather` (~3.5 ns/row vs ~1.1 us per 128-row
`indirect_dma_start`).  Its indices are int16 (15-bit usable), so the host
builds, per quarter-core (1024 items), a deduplicated sub-table of the
~23.4k unique referenced rows (< 32767) and remaps indices into it; the
device does the full 26-rows/item expansion from HBM.  One dma_gather is
capped at 1024 indices (256 B rows; the descriptor path tops out at 2^18
bytes — HW-probed), so each quarter issues 26 chunks of 1024 rows, spread
round-robin over 4 SWDGE queues so all four Q7 core-pairs generate
descriptors in parallel.  transpose=True (XBAR spray) is NOT used: under
multi-queue load its rx/tx rings desync and chunk k+1's data lands on chunk
k's columns (HW-probed, deterministic).  Plain gathers put row j in
partition j%128, block j//128, so ordering indices as j = (tile*26 + slot)
*128 + p reproduces the [item-partition, slot-block] layout directly.

Compute per 128-item tile: DVE pairwise tree (bf16, 2 elem/cyc) sums the 25
neighbour slots; PE transposes node + neighbour-sum into [dim, item] (2
transpose matmuls) and applies W_top / W_bot/25 with PSUM accumulation; ACT
does the PSUM->SBUF copy and the relu so DVE/PE stay on their critical
paths.
"""

import sys

sys.path.insert(0, "/opt/trn_rl_repo")

import numpy as np

from concourse import bacc, bass, mybir, tile
from concourse.bass_utils import run_bass_kernel_spmd
from concourse.masks import make_identity

N_NODES = 100000
DIM = 128
B = 32768
K = 25
UNITS = 128
N_CORES = 8
P = 128
IDX_W = K + 1

# gather batches: dedup scope is one batch (int16 sub-table indices).  The
# last two batches are smaller so the trailing compute backlog after the
# final gather is half a quarter, not a full one.
BATCHES = [1024, 1024, 1024, 512, 512]
T_ITEMS = 128  # items per compute tile
TI = IDX_W * T_ITEMS  # indices per tile (3328)
NI_MAX = IDX_W * max(BATCHES)  # 26624
IDX_COLS = NI_MAX // 16
U_MAX = 25000  # sub-table capacity (unique rows per batch ~23.4k +- 0.1k)
# 1024 indices (256 KB) per dma_gather is the HW cap (the descriptor path
# tops out at 2^18 bytes) and also the measured optimum: total time is
# roughly bytes/BW + ops * ~0.5 us, so fewer, larger gathers win (896 / 384
# / mixed chunk cycles all measured slower).
NI_CHUNK = 1024


def batch_chunks(items):
    ni = IDX_W * items
    return [(a, min(a + NI_CHUNK, ni)) for a in range(0, ni, NI_CHUNK)]

BF16_NP = mybir.dt.np(mybir.dt.bfloat16)


def build_program(n_items):
    assert n_items == sum(BATCHES)
    n_q = len(BATCHES)

    nc = bacc.Bacc("TRN2", target_bir_lowering=False, debug=False,
                   num_swdge_queues=2)
    f32 = mybir.dt.float32
    bf16 = mybir.dt.bfloat16
    subtab = nc.dram_tensor(
        "subtab", [n_q, U_MAX, DIM], bf16, kind="ExternalInput"
    ).ap()
    idx16 = nc.dram_tensor(
        "idx16", [n_q, P, IDX_COLS], mybir.dt.int16, kind="ExternalInput"
    ).ap()
    wt = nc.dram_tensor("wt", [DIM, UNITS], bf16, kind="ExternalInput").ap()
    wb = nc.dram_tensor("wb", [DIM, UNITS], bf16, kind="ExternalInput").ap()
    out = nc.dram_tensor("out", [n_items, UNITS], f32, kind="ExternalOutput").ap()

    relu = mybir.ActivationFunctionType.Relu
    copyf = mybir.ActivationFunctionType.Copy

    with tile.TileContext(nc) as tc:
        with (
            tc.tile_pool(name="const", bufs=1) as cpool,
            tc.tile_pool(name="gpool", bufs=2) as gpool,
            tc.tile_pool(name="spool", bufs=2) as spool,
            tc.tile_pool(name="opool", bufs=3) as opool,
            tc.tile_pool(name="pp", bufs=2, space="PSUM") as pp,
            tc.tile_pool(name="ppw", bufs=1, space="PSUM") as ppw,
        ):
            wt_sb = cpool.tile([DIM, UNITS], bf16, tag="wt")
            nc.sync.dma_start(out=wt_sb[:], in_=wt[:])
            wb_sb = cpool.tile([DIM, UNITS], bf16, tag="wb")
            nc.sync.dma_start(out=wb_sb[:], in_=wb[:])
            ident = cpool.tile([P, P], bf16, tag="ident")

            # idx_sb[p, q*IDX_COLS + c] = idx16[q, p, c]; loaded per
            # batch so the first gathers don't wait on the whole array
            idx_sb = cpool.tile([P, n_q * IDX_COLS], mybir.dt.int16, tag="idx")
            for q in range(n_q):
                nc.sync.dma_start(
                    out=idx_sb[:, q * IDX_COLS : (q + 1) * IDX_COLS],
                    in_=idx16[q],
                )

            chunk_no = 0
            for q in range(n_q):
                # plain gather of 26624 rows: row j -> partition j%128,
                # block j//128; j = (t*26 + c)*128 + p, so tile t's slot c
                # sits at gq[:, (t*26+c)*128 : +128]
                gq = gpool.tile([P, NI_MAX], bf16, tag="gq")
                for a, b in batch_chunks(BATCHES[q]):
                    n = b - a
                    nc.gpsimd.dma_gather(
                        gq[:, a:b].rearrange("p (n e) -> p n e", e=DIM),
                        subtab[q],
                        idx_sb[
                            :,
                            q * IDX_COLS + a // 16 : q * IDX_COLS + b // 16,
                        ],
                        n,
                        n,
                        DIM,
                        transpose=False,
                        single_packet=False,
                        queue_num=chunk_no % 2,
                    )
                    chunk_no += 1

                if q == 0:
                    # identity + warmups sit behind batch 0's gathers in the
                    # Pool/PE streams so the gathers start immediately
                    make_identity(nc, ident[:])
                    psum_warm_t = ppw.tile([P, UNITS], bf16, tag="warmt")
                    nc.tensor.matmul(
                        out=psum_warm_t[:], lhsT=ident[:], rhs=ident[:],
                        is_transpose=True,
                    )
                    psum_warm = ppw.tile([P, UNITS], f32, tag="warm")
                    nc.tensor.matmul(out=psum_warm[:], lhsT=wt_sb[:], rhs=wt_sb[:])
                    nc.tensor.matmul(out=psum_warm[:], lhsT=wb_sb[:], rhs=wb_sb[:])

                for t in range(BATCHES[q] // T_ITEMS):
                    base = t * TI  # element column offset of tile t
                    # DVE pairwise tree over neighbour slots 1..25
                    s12 = spool.tile([P, 12 * DIM], bf16, tag="s12")
                    nc.vector.tensor_add(
                        s12[:],
                        gq[:, base + 1 * DIM : base + 13 * DIM],
                        gq[:, base + 13 * DIM : base + 25 * DIM],
                    )
                    s6 = spool.tile([P, 6 * DIM], bf16, tag="s6")
                    nc.vector.tensor_add(
                        s6[:], s12[:, : 6 * DIM], s12[:, 6 * DIM :]
                    )
                    s3 = spool.tile([P, 3 * DIM], bf16, tag="s3")
                    nc.vector.tensor_add(
                        s3[:], s6[:, : 3 * DIM], s6[:, 3 * DIM :]
                    )
                    p1 = spool.tile([P, DIM], bf16, tag="p1")
                    nc.vector.tensor_add(
                        p1[:], s3[:, :DIM], s3[:, DIM : 2 * DIM]
                    )
                    p2 = spool.tile([P, DIM], bf16, tag="p2")
                    nc.vector.tensor_add(p2[:], p1[:], s3[:, 2 * DIM :])
                    nbs = spool.tile([P, DIM], bf16, tag="nbs")
                    nc.vector.tensor_add(
                        nbs[:], p2[:], gq[:, base + 25 * DIM : base + 26 * DIM]
                    )

                    # transpose node row and neighbour sum into [dim, item]
                    psum_t = pp.tile([P, 2 * DIM], bf16, tag="tp")
                    nc.tensor.matmul(
                        out=psum_t[:, :DIM],
                        lhsT=gq[:, base : base + DIM],
                        rhs=ident[:],
                        is_transpose=True,
                        start=True,
                        stop=True,
                    )
                    nc.tensor.matmul(
                        out=psum_t[:, DIM:],
                        lhsT=nbs[:],
                        rhs=ident[:],
                        is_transpose=True,
                        start=True,
                        stop=True,
                    )
                    catT = opool.tile([P, 2 * DIM], bf16, tag="catT")
                    nc.scalar.activation(out=catT[:], in_=psum_t[:], func=copyf)

                    psum_o = pp.tile([P, UNITS], f32, tag="o")
                    nc.tensor.matmul(
                        out=psum_o[:],
                        lhsT=catT[:, :DIM],
                        rhs=wt_sb[:],
                        start=True,
                        stop=False,
                    )
                    nc.tensor.matmul(
                        out=psum_o[:],
                        lhsT=catT[:, DIM:],
                        rhs=wb_sb[:],
                        start=False,
                        stop=True,
                    )
                    o_sb = opool.tile([P, UNITS], f32, tag="osb")
                    nc.scalar.activation(out=o_sb[:], in_=psum_o[:], func=relu)
                    row = sum(BATCHES[:q]) + t * T_ITEMS
                    nc.sync.dma_start(
                        out=out[row : row + T_ITEMS, :], in_=o_sb[:]
                    )

    nc.compile()
    return nc


_PROGRAM_CACHE = {}


def _get_program(n_items):
    if n_items not in _PROGRAM_CACHE:
        _PROGRAM_CACHE[n_items] = build_program(n_items)
    return _PROGRAM_CACHE[n_items]


def _prep_core(features_bf, idx_core):
    """Per-core host prep: dedup per quarter, build sub-tables + wrapped
    int16 tile/slot-major indices."""
    n_q = len(BATCHES)
    assert idx_core.shape[0] == sum(BATCHES)
    subtab = np.zeros((n_q, U_MAX, DIM), dtype=BF16_NP)
    idx16 = np.zeros((n_q, P, IDX_COLS), dtype=np.int16)
    off = 0
    for q, items in enumerate(BATCHES):
        sl = idx_core[off : off + items]  # [items, IDX_W]
        off += items
        uniq, inv = np.unique(sl, return_inverse=True)
        assert len(uniq) <= U_MAX, f"unique rows {len(uniq)} > {U_MAX}"
        subtab[q, : len(uniq)] = features_bf[uniq]
        inv = inv.reshape(items, IDX_W).astype(np.int16)
        # index order j = (t*IDX_W + c)*128 + p
        ordered = np.concatenate(
            [
                inv[t * T_ITEMS : (t + 1) * T_ITEMS].T.ravel()
                for t in range(items // T_ITEMS)
            ]
        )
        # wrap each gather chunk into 16 partitions (within-chunk
        # j = col*16 + p), replicate to 128
        wrapped = np.concatenate(
            [ordered[a:b].reshape(-1, 16).T for a, b in batch_chunks(items)],
            axis=1,
        )
        idx16[q, :, : wrapped.shape[1]] = np.tile(wrapped, (8, 1))
    return subtab, idx16


def _prep_inputs(features, node, neighbours, W):
    features_bf = np.asarray(features, dtype=np.float32).astype(BF16_NP)
    node = np.asarray(node, dtype=np.int32).reshape(-1, 1)
    neighbours = np.asarray(neighbours, dtype=np.int32)
    W = np.asarray(W, dtype=np.float32)
    idx_all = np.ascontiguousarray(
        np.concatenate([node, neighbours], axis=1), dtype=np.int32
    )
    wt = np.ascontiguousarray(W[:DIM]).astype(BF16_NP)
    wb = (W[DIM:].astype(np.float64) / K).astype(BF16_NP)
    return features_bf, idx_all, wt, wb


def kernel(features, node, neighbours, W, trace=False):
    features_bf, idx_all, wt, wb = _prep_inputs(features, node, neighbours, W)
    n_total = idx_all.shape[0]
    per_core = n_total // N_CORES
    nc = _get_program(per_core)
    in_maps = []
    for i in range(N_CORES):
        subtab, idx16 = _prep_core(
            features_bf, idx_all[i * per_core : (i + 1) * per_core]
        )
        in_maps.append({"subtab": subtab, "idx16": idx16, "wt": wt, "wb": wb})
    res = run_bass_kernel_spmd(nc, in_maps, list(range(N_CORES)), trace=trace)
    out = np.concatenate([res.results[i]["out"] for i in range(N_CORES)], axis=0)
    if trace:
        kernel.last_result = res
    return out


# revision 31
# speedup vs baseline: 1.7930x; 1.7930x over previous
"""Trainium2 Bass kernel for nn_MeanAggregator (GNN mean aggregation).

out[b] = relu(concat(features[node[b]], mean_k features[neighbours[b,k]]) @ W)

8 NeuronCores, data-parallel over the batch (4096 items/core).  Tolerance is
2e-2, so features/W are cast to bf16 on the host (measured end-to-end error
~2.5e-3).

Gather strategy: the only SWDGE path whose Q7 descriptor generation is
vectorized is `dma_gather` (~3.5 ns/row vs ~1.1 us per 128-row
`indirect_dma_start`).  Its indices are int16 (15-bit usable), so the host
builds, per quarter-core (1024 items), a deduplicated sub-table of the
~23.4k unique referenced rows (< 32767) and remaps indices into it; the
device does the full 26-rows/item expansion from HBM.  One dma_gather is
capped at 1024 indices (256 B rows; the descriptor path tops out at 2^18
bytes — HW-probed), so each quarter issues 26 chunks of 1024 rows, spread
round-robin over 4 SWDGE queues so all four Q7 core-pairs generate
descriptors in parallel.  transpose=True (XBAR spray) is NOT used: under
multi-queue load its rx/tx rings desync and chunk k+1's data lands on chunk
k's columns (HW-probed, deterministic).  Plain gathers put row j in
partition j%128, block j//128, so ordering indices as j = (tile*26 + slot)
*128 + p reproduces the [item-partition, slot-block] layout directly.

Compute per 128-item tile: DVE pairwise tree (bf16, 2 elem/cyc) sums the 25
neighbour slots; PE transposes node + neighbour-sum into [dim, item] (2
transpose matmuls) and applies W_top / W_bot/25 with PSUM accumulation; ACT
does the PSUM->SBUF copy and the relu so DVE/PE stay on their critical
paths.
"""

import sys

sys.path.insert(0, "/opt/trn_rl_repo")

import numpy as np

from concourse import bacc, bass, mybir, tile
from concourse.bass_utils import run_bass_kernel_spmd
from concourse.masks import make_identity

N_NODES = 100000
DIM = 128
B = 32768
K = 25
UNITS = 128
N_CORES = 8
P = 128
IDX_W = K + 1

# gather batches: dedup scope is one batch (int16 sub-table indices).  The
# last two batches are smaller so the trailing compute backlog after the
# final gather is half a quarter, not a full one.
BATCHES = [1024, 1024, 1024, 512, 512]
T_ITEMS = 128  # items per compute tile
TI = IDX_W * T_ITEMS  # indices per tile (3328)
NI_MAX = IDX_W * max(BATCHES)  # 26624
IDX_COLS = NI_MAX // 16
U_MAX = 25000  # sub-table capacity (unique rows per batch ~23.4k +- 0.1k)
# 1024 indices (256 KB) per dma_gather is the HW cap (the descriptor path
# tops out at 2^18 bytes) and also the measured optimum: total time is
# roughly bytes/BW + ops * ~0.5 us, so fewer, larger gathers win (896 / 384
# / mixed chunk cycles all measured slower).
NI_CHUNK = 1024


def batch_chunks(items):
    ni = IDX_W * items
    return [(a, min(a + NI_CHUNK, ni)) for a in range(0, ni, NI_CHUNK)]

BF16_NP = mybir.dt.np(mybir.dt.bfloat16)


def build_program(n_items):
    assert n_items == sum(BATCHES)
    n_q = len(BATCHES)

    nc = bacc.Bacc("TRN2", target_bir_lowering=False, debug=False,
                   num_swdge_queues=4)
    f32 = mybir.dt.float32
    bf16 = mybir.dt.bfloat16
    subtab = nc.dram_tensor(
        "subtab", [n_q, U_MAX, DIM], bf16, kind="ExternalInput"
    ).ap()
    idx16 = nc.dram_tensor(
        "idx16", [n_q, P, IDX_COLS], mybir.dt.int16, kind="ExternalInput"
    ).ap()
    wt = nc.dram_tensor("wt", [DIM, UNITS], bf16, kind="ExternalInput").ap()
    wb = nc.dram_tensor("wb", [DIM, UNITS], bf16, kind="ExternalInput").ap()
    out = nc.dram_tensor("out", [n_items, UNITS], f32, kind="ExternalOutput").ap()

    relu = mybir.ActivationFunctionType.Relu
    copyf = mybir.ActivationFunctionType.Copy

    with tile.TileContext(nc) as tc:
        with (
            tc.tile_pool(name="const", bufs=1) as cpool,
            tc.tile_pool(name="gpool", bufs=2) as gpool,
            tc.tile_pool(name="spool", bufs=2) as spool,
            tc.tile_pool(name="opool", bufs=3) as opool,
            tc.tile_pool(name="pp", bufs=2, space="PSUM") as pp,
            tc.tile_pool(name="ppw", bufs=1, space="PSUM") as ppw,
        ):
            wt_sb = cpool.tile([DIM, UNITS], bf16, tag="wt")
            nc.sync.dma_start(out=wt_sb[:], in_=wt[:])
            wb_sb = cpool.tile([DIM, UNITS], bf16, tag="wb")
            nc.sync.dma_start(out=wb_sb[:], in_=wb[:])
            ident = cpool.tile([P, P], bf16, tag="ident")

            # idx_sb[p, q*IDX_COLS + c] = idx16[q, p, c]; loaded per
            # batch so the first gathers don't wait on the whole array
            idx_sb = cpool.tile([P, n_q * IDX_COLS], mybir.dt.int16, tag="idx")
            for q in range(n_q):
                nc.sync.dma_start(
                    out=idx_sb[:, q * IDX_COLS : (q + 1) * IDX_COLS],
                    in_=idx16[q],
                )

            chunk_no = 0
            for q in range(n_q):
                # plain gather of 26624 rows: row j -> partition j%128,
                # block j//128; j = (t*26 + c)*128 + p, so tile t's slot c
                # sits at gq[:, (t*26+c)*128 : +128]
                gq = gpool.tile([P, NI_MAX], bf16, tag="gq")
                for a, b in batch_chunks(BATCHES[q]):
                    n = b - a
                    nc.gpsimd.dma_gather(
                        gq[:, a:b].rearrange("p (n e) -> p n e", e=DIM),
                        subtab[q],
                        idx_sb[
                            :,
                            q * IDX_COLS + a // 16 : q * IDX_COLS + b // 16,
                        ],
                        n,
                        n,
                        DIM,
                        transpose=False,
                        single_packet=False,
                        queue_num=chunk_no % 4,
                    )
                    chunk_no += 1

                if q == 0:
                    # identity + warmups sit behind batch 0's gathers in the
                    # Pool/PE streams so the gathers start immediately
                    make_identity(nc, ident[:])
                    psum_warm_t = ppw.tile([P, UNITS], bf16, tag="warmt")
                    nc.tensor.matmul(
                        out=psum_warm_t[:], lhsT=ident[:], rhs=ident[:],
                        is_transpose=True,
                    )
                    psum_warm = ppw.tile([P, UNITS], f32, tag="warm")
                    nc.tensor.matmul(out=psum_warm[:], lhsT=wt_sb[:], rhs=wt_sb[:])
                    nc.tensor.matmul(out=psum_warm[:], lhsT=wb_sb[:], rhs=wb_sb[:])

                for t in range(BATCHES[q] // T_ITEMS):
                    base = t * TI  # element column offset of tile t
                    # DVE pairwise tree over neighbour slots 1..25
                    s12 = spool.tile([P, 12 * DIM], bf16, tag="s12")
                    nc.vector.tensor_add(
                        s12[:],
                        gq[:, base + 1 * DIM : base + 13 * DIM],
                        gq[:, base + 13 * DIM : base + 25 * DIM],
                    )
                    s6 = spool.tile([P, 6 * DIM], bf16, tag="s6")
                    nc.vector.tensor_add(
                        s6[:], s12[:, : 6 * DIM], s12[:, 6 * DIM :]
                    )
                    s3 = spool.tile([P, 3 * DIM], bf16, tag="s3")
                    nc.vector.tensor_add(
                        s3[:], s6[:, : 3 * DIM], s6[:, 3 * DIM :]
                    )
                    p1 = spool.tile([P, DIM], bf16, tag="p1")
                    nc.vector.tensor_add(
                        p1[:], s3[:, :DIM], s3[:, DIM : 2 * DIM]
                    )
                    p2 = spool.tile([P, DIM], bf16, tag="p2")
                    nc.vector.tensor_add(p2[:], p1[:], s3[:, 2 * DIM :])
                    nbs = spool.tile([P, DIM], bf16, tag="nbs")
                    nc.vector.tensor_add(
                        nbs[:], p2[:], gq[:, base + 25 * DIM : base + 26 * DIM]
                    )

                    # transpose node row and neighbour sum into [dim, item]
                    psum_t = pp.tile([P, 2 * DIM], bf16, tag="tp")
                    nc.tensor.matmul(
                        out=psum_t[:, :DIM],
                        lhsT=gq[:, base : base + DIM],
                        rhs=ident[:],
                        is_transpose=True,
                        start=True,
                        stop=True,
                    )
                    nc.tensor.matmul(
                        out=psum_t[:, DIM:],
                        lhsT=nbs[:],
                        rhs=ident[:],
                        is_transpose=True,
                        start=True,
                        stop=True,
                    )
                    catT = opool.tile([P, 2 * DIM], bf16, tag="catT")
                    nc.scalar.activation(out=catT[:], in_=psum_t[:], func=copyf)

                    psum_o = pp.tile([P, UNITS], f32, tag="o")
                    nc.tensor.matmul(
                        out=psum_o[:],
                        lhsT=catT[:, :DIM],
                        rhs=wt_sb[:],
                        start=True,
                        stop=False,
                    )
                    nc.tensor.matmul(
                        out=psum_o[:],
                        lhsT=catT[:, DIM:],
                        rhs=wb_sb[:],
                        start=False,
                        stop=True,
                    )
                    o_sb = opool.tile([P, UNITS], f32, tag="osb")
                    nc.scalar.activation(out=o_sb[:], in_=psum_o[:], func=relu)
                    row = sum(BATCHES[:q]) + t * T_ITEMS
                    nc.sync.dma_start(
                        out=out[row : row + T_ITEMS, :], in_=o_sb[:]
                    )

    nc.compile()
    return nc


_PROGRAM_CACHE = {}


def _get_program(n_items):
    if n_items not in _PROGRAM_CACHE:
        _PROGRAM_CACHE[n_items] = build_program(n_items)
    return _PROGRAM_CACHE[n_items]


def _prep_core(features_bf, idx_core):
    """Per-core host prep: dedup per quarter, build sub-tables + wrapped
    int16 tile/slot-major indices."""
    n_q = len(BATCHES)
    assert idx_core.shape[0] == sum(BATCHES)
    subtab = np.zeros((n_q, U_MAX, DIM), dtype=BF16_NP)
    idx16 = np.zeros((n_q, P, IDX_COLS), dtype=np.int16)
    off = 0
    for q, items in enumerate(BATCHES):
        sl = idx_core[off : off + items]  # [items, IDX_W]
        off += items
        uniq, inv = np.unique(sl, return_inverse=True)
        assert len(uniq) <= U_MAX, f"unique rows {len(uniq)} > {U_MAX}"
        subtab[q, : len(uniq)] = features_bf[uniq]
        inv = inv.reshape(items, IDX_W).astype(np.int16)
        # index order j = (t*IDX_W + c)*128 + p
        ordered = np.concatenate(
            [
                inv[t * T_ITEMS : (t + 1) * T_ITEMS].T.ravel()
                for t in range(items // T_ITEMS)
            ]
        )
        # wrap each gather chunk into 16 partitions (within-chunk
        # j = col*16 + p), replicate to 128
        wrapped = np.concatenate(
            [ordered[a:b].reshape(-1, 16).T for a, b in batch_chunks(items)],
            axis=1,
        )
        idx16[q, :, : wrapped.shape[1]] = np.tile(wrapped, (8, 1))
    return subtab, idx16


def _prep_inputs(features, node, neighbours, W):
    features_bf = np.asarray(features, dtype=np.float32).astype(BF16_NP)
    node = np.asarray(node, dtype=np.int32).reshape(-1, 1)
    neighbours = np.asarray(neighbours, dtype=np.int32)
    W = np.asarray(W, dtype=np.float32)
    idx_all = np.ascontiguousarray(
        np.concatenate([node, neighbours], axis=1), dtype=np.int32
    )
    wt = np.ascontiguousarray(W[:DIM]).astype(BF16_NP)
    wb = (W[DIM:].astype(np.float64) / K).astype(BF16_NP)
    return features_bf, idx_all, wt, wb


def kernel(features, node, neighbours, W, trace=False):
    features_bf, idx_all, wt, wb = _prep_inputs(features, node, neighbours, W)
    n_total = idx_all.shape[0]
    per_core = n_total // N_CORES
    nc = _get_program(per_core)
    in_maps = []
    for i in range(N_CORES):
        subtab, idx16 = _prep_core(
            features_bf, idx_all[i * per_core : (i + 1) * per_core]
        )
        in_maps.append({"subtab": subtab, "idx16": idx16, "wt": wt, "wb": wb})
    res = run_bass_kernel_spmd(nc, in_maps, list(range(N_CORES)), trace=trace)
    out = np.concatenate([res.results[i]["out"] for i in range(N_CORES)], axis=0)
    if trace:
        kernel.last_result = res
    return out


# revision 32
# speedup vs baseline: 1.7939x; 1.0005x over previous
"""Trainium2 Bass kernel for nn_MeanAggregator (GNN mean aggregation).

out[b] = relu(concat(features[node[b]], mean_k features[neighbours[b,k]]) @ W)

8 NeuronCores, data-parallel over the batch (4096 items/core).  Tolerance is
2e-2, so features/W are cast to bf16 on the host (measured end-to-end error
~2.5e-3).

Gather strategy: the only SWDGE path whose Q7 descriptor generation is
vectorized is `dma_gather` (~3.5 ns/row vs ~1.1 us per 128-row
`indirect_dma_start`).  Its indices are int16 (15-bit usable), so the host
builds, per gather batch (<= 1024 items), a deduplicated sub-table of the
~23.4k unique referenced rows (< 32767) and remaps indices into it; the
device does the full 26-rows/item expansion from HBM.  One dma_gather is
capped at 1024 indices (= 65 SWDGE ring entries — HW-probed: 1280 wedges
the device), so each batch issues ceil(26*items/1024) chunks, spread
round-robin over 4 SWDGE queues so all four Q7 core-pairs generate
descriptors in parallel.  transpose=True (XBAR spray) is NOT used: under
multi-queue load its rx/tx rings desync and chunk k+1's data lands on chunk
k's columns (HW-probed, deterministic).  Plain gathers put row j in
partition j%128, block j//128, so ordering indices as j = (tile*26 + slot)
*128 + p reproduces the [item-partition, slot-block] layout directly.

Compute per 128-item tile: DVE pairwise tree (bf16, 2 elem/cyc) sums the 25
neighbour slots; PE transposes node + neighbour-sum into [dim, item] (2
transpose matmuls) and applies W_top / W_bot/25 with PSUM accumulation; ACT
does the PSUM->SBUF copy and the relu so DVE/PE stay on their critical
paths.
"""

import sys

sys.path.insert(0, "/opt/trn_rl_repo")

import numpy as np

from concourse import bacc, bass, mybir, tile
from concourse.bass_utils import run_bass_kernel_spmd
from concourse.masks import make_identity

N_NODES = 100000
DIM = 128
B = 32768
K = 25
UNITS = 128
N_CORES = 8
P = 128
IDX_W = K + 1

# gather batches: dedup scope is one batch (int16 sub-table indices).  The
# last two batches are smaller so the trailing compute backlog after the
# final gather is half a quarter, not a full one.
BATCHES = [1024, 1024, 1024, 512, 512]
T_ITEMS = 128  # items per compute tile
TI = IDX_W * T_ITEMS  # indices per tile (3328)
NI_MAX = IDX_W * max(BATCHES)  # 26624
IDX_COLS = NI_MAX // 16
U_MAX = 25000  # sub-table capacity (unique rows per batch ~23.4k +- 0.1k)
# 1024 indices (256 KB) per dma_gather is the HW cap (the descriptor path
# tops out at 2^18 bytes) and also the measured optimum: total time is
# roughly bytes/BW + ops * ~0.5 us, so fewer, larger gathers win (896 / 384
# / mixed chunk cycles all measured slower).
NI_CHUNK = 1024


def batch_chunks(items):
    ni = IDX_W * items
    return [(a, min(a + NI_CHUNK, ni)) for a in range(0, ni, NI_CHUNK)]

BF16_NP = mybir.dt.np(mybir.dt.bfloat16)


def build_program(n_items):
    assert n_items == sum(BATCHES)
    n_q = len(BATCHES)

    nc = bacc.Bacc("TRN2", target_bir_lowering=False, debug=False,
                   num_swdge_queues=4)
    f32 = mybir.dt.float32
    bf16 = mybir.dt.bfloat16
    subtab = nc.dram_tensor(
        "subtab", [n_q, U_MAX, DIM], bf16, kind="ExternalInput"
    ).ap()
    idx16 = nc.dram_tensor(
        "idx16", [n_q, P, IDX_COLS], mybir.dt.int16, kind="ExternalInput"
    ).ap()
    wt = nc.dram_tensor("wt", [DIM, UNITS], bf16, kind="ExternalInput").ap()
    wb = nc.dram_tensor("wb", [DIM, UNITS], bf16, kind="ExternalInput").ap()
    out = nc.dram_tensor("out", [n_items, UNITS], f32, kind="ExternalOutput").ap()

    relu = mybir.ActivationFunctionType.Relu
    copyf = mybir.ActivationFunctionType.Copy

    with tile.TileContext(nc) as tc:
        with (
            tc.tile_pool(name="const", bufs=1) as cpool,
            tc.tile_pool(name="gpool", bufs=2) as gpool,
            tc.tile_pool(name="spool", bufs=2) as spool,
            tc.tile_pool(name="opool", bufs=3) as opool,
            tc.tile_pool(name="pp", bufs=2, space="PSUM") as pp,
            tc.tile_pool(name="ppw", bufs=1, space="PSUM") as ppw,
        ):
            wt_sb = cpool.tile([DIM, UNITS], bf16, tag="wt")
            nc.sync.dma_start(out=wt_sb[:], in_=wt[:])
            wb_sb = cpool.tile([DIM, UNITS], bf16, tag="wb")
            nc.sync.dma_start(out=wb_sb[:], in_=wb[:])
            ident = cpool.tile([P, P], bf16, tag="ident")

            # idx_sb[p, q*IDX_COLS + c] = idx16[q, p, c]; loaded per
            # batch so the first gathers don't wait on the whole array
            idx_sb = cpool.tile([P, n_q * IDX_COLS], mybir.dt.int16, tag="idx")
            for q in range(n_q):
                nc.sync.dma_start(
                    out=idx_sb[:, q * IDX_COLS : (q + 1) * IDX_COLS],
                    in_=idx16[q],
                )

            chunk_no = 0
            for q in range(n_q):
                # plain gather of 26624 rows: row j -> partition j%128,
                # block j//128; j = (t*26 + c)*128 + p, so tile t's slot c
                # sits at gq[:, (t*26+c)*128 : +128]
                gq = gpool.tile([P, NI_MAX], bf16, tag="gq")
                for a, b in batch_chunks(BATCHES[q]):
                    n = b - a
                    nc.gpsimd.dma_gather(
                        gq[:, a:b].rearrange("p (n e) -> p n e", e=DIM),
                        subtab[q],
                        idx_sb[
                            :,
                            q * IDX_COLS + a // 16 : q * IDX_COLS + b // 16,
                        ],
                        n,
                        n,
                        DIM,
                        transpose=False,
                        single_packet=False,
                        queue_num=chunk_no % 4,
                    )
                    chunk_no += 1

                if q == 0:
                    # identity + warmups sit behind batch 0's gathers in the
                    # Pool/PE streams so the gathers start immediately
                    make_identity(nc, ident[:])
                    psum_warm_t = ppw.tile([P, UNITS], bf16, tag="warmt")
                    nc.tensor.matmul(
                        out=psum_warm_t[:], lhsT=ident[:], rhs=ident[:],
                        is_transpose=True,
                    )
                    psum_warm = ppw.tile([P, UNITS], f32, tag="warm")
                    nc.tensor.matmul(out=psum_warm[:], lhsT=wt_sb[:], rhs=wt_sb[:])
                    nc.tensor.matmul(out=psum_warm[:], lhsT=wb_sb[:], rhs=wb_sb[:])

                for t in range(BATCHES[q] // T_ITEMS):
                    base = t * TI  # element column offset of tile t
                    # DVE pairwise tree over neighbour slots 1..25
                    s12 = spool.tile([P, 12 * DIM], bf16, tag="s12")
                    nc.vector.tensor_add(
                        s12[:],
                        gq[:, base + 1 * DIM : base + 13 * DIM],
                        gq[:, base + 13 * DIM : base + 25 * DIM],
                    )
                    s6 = spool.tile([P, 6 * DIM], bf16, tag="s6")
                    nc.vector.tensor_add(
                        s6[:], s12[:, : 6 * DIM], s12[:, 6 * DIM :]
                    )
                    s3 = spool.tile([P, 3 * DIM], bf16, tag="s3")
                    nc.vector.tensor_add(
                        s3[:], s6[:, : 3 * DIM], s6[:, 3 * DIM :]
                    )
                    p1 = spool.tile([P, DIM], bf16, tag="p1")
                    nc.vector.tensor_add(
                        p1[:], s3[:, :DIM], s3[:, DIM : 2 * DIM]
                    )
                    p2 = spool.tile([P, DIM], bf16, tag="p2")
                    nc.vector.tensor_add(p2[:], p1[:], s3[:, 2 * DIM :])
                    nbs = spool.tile([P, DIM], bf16, tag="nbs")
                    nc.vector.tensor_add(
                        nbs[:], p2[:], gq[:, base + 25 * DIM : base + 26 * DIM]
                    )

                    # transpose node row and neighbour sum into [dim, item]
                    psum_t = pp.tile([P, 2 * DIM], bf16, tag="tp")
                    nc.tensor.matmul(
                        out=psum_t[:, :DIM],
                        lhsT=gq[:, base : base + DIM],
                        rhs=ident[:],
                        is_transpose=True,
                        start=True,
                        stop=True,
                    )
                    nc.tensor.matmul(
                        out=psum_t[:, DIM:],
                        lhsT=nbs[:],
                        rhs=ident[:],
                        is_transpose=True,
                        start=True,
                        stop=True,
                    )
                    catT = opool.tile([P, 2 * DIM], bf16, tag="catT")
                    nc.scalar.activation(out=catT[:], in_=psum_t[:], func=copyf)

                    psum_o = pp.tile([P, UNITS], f32, tag="o")
                    nc.tensor.matmul(
                        out=psum_o[:],
                        lhsT=catT[:, :DIM],
                        rhs=wt_sb[:],
                        start=True,
                        stop=False,
                    )
                    nc.tensor.matmul(
                        out=psum_o[:],
                        lhsT=catT[:, DIM:],
                        rhs=wb_sb[:],
                        start=False,
                        stop=True,
                    )
                    o_sb = opool.tile([P, UNITS], f32, tag="osb")
                    nc.scalar.activation(out=o_sb[:], in_=psum_o[:], func=relu)
                    row = sum(BATCHES[:q]) + t * T_ITEMS
                    nc.sync.dma_start(
                        out=out[row : row + T_ITEMS, :], in_=o_sb[:]
                    )

    nc.compile()
    return nc


_PROGRAM_CACHE = {}


def _get_program(n_items):
    if n_items not in _PROGRAM_CACHE:
        _PROGRAM_CACHE[n_items] = build_program(n_items)
    return _PROGRAM_CACHE[n_items]


def _prep_core(features_bf, idx_core):
    """Per-core host prep: dedup per quarter, build sub-tables + wrapped
    int16 tile/slot-major indices."""
    n_q = len(BATCHES)
    assert idx_core.shape[0] == sum(BATCHES)
    subtab = np.zeros((n_q, U_MAX, DIM), dtype=BF16_NP)
    idx16 = np.zeros((n_q, P, IDX_COLS), dtype=np.int16)
    off = 0
    for q, items in enumerate(BATCHES):
        sl = idx_core[off : off + items]  # [items, IDX_W]
        off += items
        uniq, inv = np.unique(sl, return_inverse=True)
        assert len(uniq) <= U_MAX, f"unique rows {len(uniq)} > {U_MAX}"
        subtab[q, : len(uniq)] = features_bf[uniq]
        inv = inv.reshape(items, IDX_W).astype(np.int16)
        # index order j = (t*IDX_W + c)*128 + p
        ordered = np.concatenate(
            [
                inv[t * T_ITEMS : (t + 1) * T_ITEMS].T.ravel()
                for t in range(items // T_ITEMS)
            ]
        )
        # wrap each gather chunk into 16 partitions (within-chunk
        # j = col*16 + p), replicate to 128
        wrapped = np.concatenate(
            [ordered[a:b].reshape(-1, 16).T for a, b in batch_chunks(items)],
            axis=1,
        )
        idx16[q, :, : wrapped.shape[1]] = np.tile(wrapped, (8, 1))
    return subtab, idx16


def _prep_inputs(features, node, neighbours, W):
    features_bf = np.asarray(features, dtype=np.float32).astype(BF16_NP)
    node = np.asarray(node, dtype=np.int32).reshape(-1, 1)
    neighbours = np.asarray(neighbours, dtype=np.int32)
    W = np.asarray(W, dtype=np.float32)
    idx_all = np.ascontiguousarray(
        np.concatenate([node, neighbours], axis=1), dtype=np.int32
    )
    wt = np.ascontiguousarray(W[:DIM]).astype(BF16_NP)
    wb = (W[DIM:].astype(np.float64) / K).astype(BF16_NP)
    return features_bf, idx_all, wt, wb


def kernel(features, node, neighbours, W, trace=False):
    features_bf, idx_all, wt, wb = _prep_inputs(features, node, neighbours, W)
    n_total = idx_all.shape[0]
    per_core = n_total // N_CORES
    nc = _get_program(per_core)
    in_maps = []
    for i in range(N_CORES):
        subtab, idx16 = _prep_core(
            features_bf, idx_all[i * per_core : (i + 1) * per_core]
        )
        in_maps.append({"subtab": subtab, "idx16": idx16, "wt": wt, "wb": wb})
    res = run_bass_kernel_spmd(nc, in_maps, list(range(N_CORES)), trace=trace)
    out = np.concatenate([res.results[i]["out"] for i in range(N_CORES)], axis=0)
    if trace:
        kernel.last_result = res
    return out
